# revision 5
# baseline (speedup 1.0000x reference)
"""Bass/Trainium2 kernel for nn_ContrastiveLoss (8-core SPMD).

Math (matching the reference):
    S_xy = exp(x @ yf.T / TEMP)   [N, T*Q]   yf = y.reshape(T*Q, d)
    S_xx = exp(x @ x.T / TEMP)    [N, N]
    per-row scalars:
      rxy_total[n] = sum_m S_xy[n, m]
      rxy_pos[n]   = sum_m S_xy[n, m] * (m % T == tid[n])
      rxx_total[n] = sum_m S_xx[n, m]
      rxx_posf[n]  = sum_m S_xx[n, m] * (tid[m] == tid[n])
    num_i = rxy_pos + 0.5*(rxx_posf - diag)     diag = exp(||x_n||^2/TEMP)
    den_i = (rxy_total - rxy_pos) + (rxx_total - rxx_posf)
    then a T-sized segment sum over tid and the log-ratio mean (host).

Sharding over 8 cores:
  - xy stage: column-parallel. Core c owns yf rows [c*2048, (c+1)*2048) and
    computes partial rxy_total / rxy_pos for ALL 2048 x-rows; partials are
    summed on the host. (m % T) for core c's local column k is k % T since
    2048 is a multiple of T=256.
  - xx stage: row-parallel. Core c owns x rows [c*256, (c+1)*256) and
    computes full rxx_total / rxx_posf for those rows against all of x.

Device kernel per core:
  - matmul (fp32, K=256 split in 2 chunks accumulated in PSUM, N blocks of 512)
  - exp on ScalarE straight out of PSUM with accum_out = row totals
  - masked "positive" sums via one fused tensor_tensor_reduce per row-group
    (one-hot track mask broadcast across the 64/8 q-repeats with a step-0 AP)
"""

import numpy as np
from contextlib import ExitStack

import concourse.bass as bass
import concourse.bacc as bacc
import concourse.mybir as mybir
import concourse.tile as tile
from concourse import bass_utils

N, D, T, Q = 2048, 256, 256, 64
TEMP = 0.3
NCORES = 8
R = N // NCORES            # 256 x-rows per core (xx stage)
YC = (T * Q) // NCORES     # 2048 y-cols per core (xy stage)
NRG = N // 128             # 16 row groups (xy stage)
NXG = R // 128             # 2 row groups (xx stage)
KC = D // 128              # 2 contraction chunks
NB = 512                   # matmul moving-operand block (one PSUM bank)

F32 = mybir.dt.float32
AF = mybir.ActivationFunctionType
ALU = mybir.AluOpType


def _build_bass():
    nc = bacc.Bacc(
        "TRN2",
        target_bir_lowering=False,
        debug=False,
        enable_asserts=False,
        num_devices=NCORES,
    )
    xt = nc.dram_tensor("xt", [KC, 128, N], F32, kind="ExternalInput").ap()
    xtl = nc.dram_tensor("xtl", [KC, 128, R], F32, kind="ExternalInput").ap()
    yt = nc.dram_tensor("yt", [KC, 128, YC], F32, kind="ExternalInput").ap()
    oh = nc.dram_tensor("oh", [NRG, 128, T], F32, kind="ExternalInput").ap()
    mmk = nc.dram_tensor("mmk", [NXG, 128, N], F32, kind="ExternalInput").ap()
    out = nc.dram_tensor("out", [128, 2 * NRG + 2 * NXG], F32, kind="ExternalOutput").ap()

    with tile.TileContext(nc) as tc:
        _kernel(tc, out, xt, xtl, yt, oh, mmk)
    nc.compile()
    return nc


def _kernel(tc, out, xt, xtl, yt, oh, mmk):
    nc = tc.nc
    with ExitStack() as ctx:
        const = ctx.enter_context(tc.tile_pool(name="const", bufs=1))
        psum = ctx.enter_context(tc.tile_pool(name="psum", bufs=2, space="PSUM"))
        sp = ctx.enter_context(tc.tile_pool(name="sp", bufs=2))
        scrp = ctx.enter_context(tc.tile_pool(name="scrp", bufs=2))

        xt_sb = []
        yt_sb = []
        xtl_sb = []
        for k in range(KC):
            xk = const.tile([128, N], F32, name=f"xt_sb{k}", tag=f"xt{k}")
            nc.sync.dma_start(out=xk, in_=xt[k])
            xt_sb.append(xk)
            yk = const.tile([128, YC], F32, name=f"yt_sb{k}", tag=f"yt{k}")
            nc.sync.dma_start(out=yk, in_=yt[k])
            yt_sb.append(yk)
            xlk = const.tile([128, R], F32, name=f"xtl_sb{k}", tag=f"xtl{k}")
            nc.sync.dma_start(out=xlk, in_=xtl[k])
            xtl_sb.append(xlk)

        oh_sb = const.tile([128, NRG, T], F32, name="oh_sb")
        nc.sync.dma_start(out=oh_sb, in_=oh.transpose([1, 0, 2]))
        mm_sb = const.tile([128, NXG, N], F32, name="mm_sb")
        nc.sync.dma_start(out=mm_sb, in_=mmk.transpose([1, 0, 2]))

        pos_acc = const.tile([128, NRG], F32, name="pos_acc")
        tot_acc = const.tile([128, NRG], F32, name="tot_acc")
        xxp_acc = const.tile([128, NXG], F32, name="xxp_acc")
        xxt_acc = const.tile([128, NXG], F32, name="xxt_acc")

        # ---- xy stage: all x rows vs this core's y columns ----
        for rg in range(NRG):
            pt = psum.tile([128, YC], F32, name="pt_xy", tag="pt")
            for k in range(KC):
                lhsT = xt_sb[k][:, rg * 128:(rg + 1) * 128]
                for cb in range(YC // NB):
                    nc.tensor.matmul(
                        pt[:, cb * NB:(cb + 1) * NB],
                        lhsT,
                        yt_sb[k][:, cb * NB:(cb + 1) * NB],
                        start=(k == 0),
                        stop=(k == KC - 1),
                    )
            s = sp.tile([128, YC], F32, name="s_xy", tag="s")
            nc.scalar.activation(
                out=s, in_=pt, func=AF.Exp, scale=1.0 / TEMP,
                accum_out=tot_acc[:, rg:rg + 1],
            )
            scr = scrp.tile([128, YC], F32, name="scr_xy", tag="scr")
            nc.vector.affine_mul_reduce(
                out=scr.rearrange("p (j t) -> p j t", t=T),
                accum_out=pos_acc[:, rg:rg + 1],
                in0=s.rearrange("p (j t) -> p j t", t=T),
                in1=oh_sb[:, rg, :].unsqueeze(1).broadcast_to((128, YC // T, T)),
                scale=1.0,
                bias=0.0,
            )

        # ---- xx stage: this core's x rows vs all x columns ----
        for g in range(NXG):
            pt = psum.tile([128, N], F32, name="pt_xx", tag="pt")
            for k in range(KC):
                lhsT = xtl_sb[k][:, g * 128:(g + 1) * 128]
                for cb in range(N // NB):
                    nc.tensor.matmul(
                        pt[:, cb * NB:(cb + 1) * NB],
                        lhsT,
                        xt_sb[k][:, cb * NB:(cb + 1) * NB],
                        start=(k == 0),
                        stop=(k == KC - 1),
                    )
            s = sp.tile([128, N], F32, name="s_xx", tag="s")
            nc.scalar.activation(
                out=s, in_=pt, func=AF.Exp, scale=1.0 / TEMP,
                accum_out=xxt_acc[:, g:g + 1],
            )
            scr = scrp.tile([128, N], F32, name="scr_xx", tag="scr")
            nc.vector.affine_mul_reduce(
                out=scr,
                accum_out=xxp_acc[:, g:g + 1],
                in0=s,
                in1=mm_sb[:, g, :],
                scale=1.0,
                bias=0.0,
            )

        nc.sync.dma_start(out=out[:, 0:NRG], in_=pos_acc)
        nc.sync.dma_start(out=out[:, NRG:2 * NRG], in_=tot_acc)
        nc.sync.dma_start(out=out[:, 2 * NRG:2 * NRG + NXG], in_=xxp_acc)
        nc.sync.dma_start(out=out[:, 2 * NRG + NXG:2 * NRG + 2 * NXG], in_=xxt_acc)


def make_in_maps(x, tid):
    """Per-core input dicts. x: [N, D] f32; tid: [N] int."""
    xt3 = np.ascontiguousarray(x.T).reshape(KC, 128, N)
    oh = np.zeros((N, T), np.float32)
    oh[np.arange(N), tid] = 1.0
    oh3 = np.ascontiguousarray(oh.reshape(NRG, 128, T))

    in_maps = []
    for c in range(NCORES):
        xtl = np.ascontiguousarray(xt3[:, :, c * R:(c + 1) * R])
        rows = tid[c * R:(c + 1) * R]
        mm = (rows[:, None] == tid[None, :]).astype(np.float32)
        mm3 = np.ascontiguousarray(mm.reshape(NXG, 128, N))
        in_maps.append({
            "xt": xt3,
            "xtl": xtl,
            "yt": None,  # filled below (depends on y)
            "oh": oh3,
            "mmk": mm3,
        })
    return in_maps


def fill_y_slices(in_maps, y):
    yf = np.ascontiguousarray(y.reshape(T * Q, D))
    for c in range(NCORES):
        ys = yf[c * YC:(c + 1) * YC]          # [YC, D]
        in_maps[c]["yt"] = np.ascontiguousarray(ys.T).reshape(KC, 128, YC)


def combine(outs, x, tid):
    """outs: list of per-core 'out' arrays [128, 36]. Returns loss [1] f32."""
    rxy_pos = np.zeros(N, np.float64)
    rxy_tot = np.zeros(N, np.float64)
    rxx_posf = np.zeros(N, np.float64)
    rxx_tot = np.zeros(N, np.float64)
    for c, o in enumerate(outs):
        o = o.astype(np.float64)
        # xy partials cover all rows; row of (p, rg) is 128*rg + p
        rxy_pos += o[:, 0:NRG].T.reshape(N)
        rxy_tot += o[:, NRG:2 * NRG].T.reshape(N)
        # xx covers this core's rows only
        rxx_posf[c * R:(c + 1) * R] = o[:, 2 * NRG:2 * NRG + NXG].T.reshape(R)
        rxx_tot[c * R:(c + 1) * R] = o[:, 2 * NRG + NXG:2 * NRG + 2 * NXG].T.reshape(R)

    xd = x.astype(np.float64)
    diag = np.exp((xd * xd).sum(axis=1) / TEMP)

    num_i = rxy_pos + 0.5 * (rxx_posf - diag)
    den_i = (rxy_tot - rxy_pos) + (rxx_tot - rxx_posf)

    num_t = np.bincount(tid, weights=num_i, minlength=T)
    den_t = np.bincount(tid, weights=den_i, minlength=T)
    counts = np.bincount(tid, minlength=T)
    present = counts > 0
    loss_t = -np.log(num_t[present] / (den_t[present] + num_t[present]))
    loss = loss_t.sum() / present.sum()
    return np.asarray([loss], dtype=np.float32)


_NC_CACHE = None


def _get_nc():
    global _NC_CACHE
    if _NC_CACHE is None:
        _NC_CACHE = _build_bass()
    return _NC_CACHE


def kernel(x, track_idxs, y, _trace=False):
    x = np.ascontiguousarray(np.asarray(x), dtype=np.float32)
    y = np.ascontiguousarray(np.asarray(y), dtype=np.float32)
    tid = np.asarray(track_idxs).astype(np.int64)

    nc = _get_nc()
    in_maps = make_in_maps(x, tid)
    fill_y_slices(in_maps, y)

    res = bass_utils.run_bass_kernel_spmd(
        nc, in_maps, core_ids=list(range(NCORES)), trace=_trace,
    )
    outs = [r["out"] for r in res.results]
    loss = combine(outs, x, tid)
    if _trace:
        return loss, res
    return loss


# revision 6
# speedup vs baseline: 2.2024x; 2.2024x over previous
"""Bass/Trainium2 kernel for nn_ContrastiveLoss (8-core SPMD).

Math (matching the reference):
    S_xy = exp(x @ yf.T / TEMP)   [N, T*Q]   yf = y.reshape(T*Q, d)
    S_xx = exp(x @ x.T / TEMP)    [N, N]
    per-row scalars:
      rxy_total[n] = sum_m S_xy[n, m]
      rxy_pos[n]   = sum_m S_xy[n, m] * (m % T == tid[n])
      rxx_total[n] = sum_m S_xx[n, m]
      rxx_posf[n]  = sum_m S_xx[n, m] * (tid[m] == tid[n])
    num_i = rxy_pos + 0.5*(rxx_posf - diag)     diag = exp(||x_n||^2/TEMP)
    den_i = (rxy_total - rxy_pos) + (rxx_total - rxx_posf)
    then a T-sized segment sum over tid and the log-ratio mean (host).

Sharding over 8 cores:
  - xy stage: column-parallel. Core c owns yf rows [c*2048, (c+1)*2048) and
    computes partial rxy_total / rxy_pos for ALL 2048 x-rows; partials are
    summed on the host. (m % T) for core c's local column k is k % T since
    2048 is a multiple of T=256.
  - xx stage: row-parallel. Core c owns x rows [c*256, (c+1)*256) and
    computes full rxx_total / rxx_posf for those rows against all of x.

Device kernel per core:
  - matmul (fp32, K=256 split in 2 chunks accumulated in PSUM, N blocks of 512)
  - exp on ScalarE straight out of PSUM with accum_out = row totals
  - masked "positive" sums via one fused tensor_tensor_reduce per row-group
    (one-hot track mask broadcast across the 64/8 q-repeats with a step-0 AP)
"""

import numpy as np
from contextlib import ExitStack

import concourse.bass as bass
import concourse.bacc as bacc
import concourse.mybir as mybir
import concourse.tile as tile
from concourse import bass_utils

N, D, T, Q = 2048, 256, 256, 64
TEMP = 0.3
NCORES = 8
R = N // NCORES            # 256 x-rows per core (xx stage)
YC = (T * Q) // NCORES     # 2048 y-cols per core (xy stage)
NRG = N // 128             # 16 row groups (xy stage)
NXG = R // 128             # 2 row groups (xx stage)
KC = D // 128              # 2 contraction chunks
NB = 512                   # matmul moving-operand block (one PSUM bank)

F32 = mybir.dt.float32
BF16 = mybir.dt.bfloat16
AF = mybir.ActivationFunctionType
ALU = mybir.AluOpType


def _build_bass():
    nc = bacc.Bacc(
        "TRN2",
        target_bir_lowering=False,
        debug=False,
        enable_asserts=False,
        num_devices=NCORES,
    )
    xt = nc.dram_tensor("xt", [KC, 128, N], BF16, kind="ExternalInput").ap()
    xtl = nc.dram_tensor("xtl", [KC, 128, R], BF16, kind="ExternalInput").ap()
    yt = nc.dram_tensor("yt", [KC, 128, YC], BF16, kind="ExternalInput").ap()
    oh = nc.dram_tensor("oh", [NRG, 128, T], F32, kind="ExternalInput").ap()
    mmk = nc.dram_tensor("mmk", [NXG, 128, N], F32, kind="ExternalInput").ap()
    out = nc.dram_tensor("out", [128, 2 * NRG + 2 * NXG], F32, kind="ExternalOutput").ap()

    with tile.TileContext(nc) as tc:
        _kernel(tc, out, xt, xtl, yt, oh, mmk)
    nc.compile()
    return nc


def _kernel(tc, out, xt, xtl, yt, oh, mmk):
    nc = tc.nc
    with ExitStack() as ctx:
        const = ctx.enter_context(tc.tile_pool(name="const", bufs=1))
        psum = ctx.enter_context(tc.tile_pool(name="psum", bufs=2, space="PSUM"))
        sp = ctx.enter_context(tc.tile_pool(name="sp", bufs=2))
        scrp = ctx.enter_context(tc.tile_pool(name="scrp", bufs=2))

        xt_sb = []
        yt_sb = []
        xtl_sb = []
        for k in range(KC):
            xk = const.tile([128, N], BF16, name=f"xt_sb{k}", tag=f"xt{k}")
            nc.sync.dma_start(out=xk, in_=xt[k])
            xt_sb.append(xk)
            yk = const.tile([128, YC], BF16, name=f"yt_sb{k}", tag=f"yt{k}")
            nc.sync.dma_start(out=yk, in_=yt[k])
            yt_sb.append(yk)
            xlk = const.tile([128, R], BF16, name=f"xtl_sb{k}", tag=f"xtl{k}")
            nc.sync.dma_start(out=xlk, in_=xtl[k])
            xtl_sb.append(xlk)

        oh_sb = const.tile([128, NRG, T], F32, name="oh_sb")
        nc.sync.dma_start(out=oh_sb, in_=oh.transpose([1, 0, 2]))
        mm_sb = const.tile([128, NXG, N], F32, name="mm_sb")
        nc.sync.dma_start(out=mm_sb, in_=mmk.transpose([1, 0, 2]))

        pos_acc = const.tile([128, NRG], F32, name="pos_acc")
        tot_acc = const.tile([128, NRG], F32, name="tot_acc")
        xxp_acc = const.tile([128, NXG], F32, name="xxp_acc")
        xxt_acc = const.tile([128, NXG], F32, name="xxt_acc")

        # ---- xy stage: all x rows vs this core's y columns ----
        for rg in range(NRG):
            pt = psum.tile([128, YC], F32, name="pt_xy", tag="pt")
            for k in range(KC):
                lhsT = xt_sb[k][:, rg * 128:(rg + 1) * 128]
                for cb in range(YC // NB):
                    nc.tensor.matmul(
                        pt[:, cb * NB:(cb + 1) * NB],
                        lhsT,
                        yt_sb[k][:, cb * NB:(cb + 1) * NB],
                        start=(k == 0),
                        stop=(k == KC - 1),
                    )
            s = sp.tile([128, YC], F32, name="s_xy", tag="s")
            nc.scalar.activation(
                out=s, in_=pt, func=AF.Exp, scale=1.0 / TEMP,
                accum_out=tot_acc[:, rg:rg + 1],
            )
            scr = scrp.tile([128, YC], F32, name="scr_xy", tag="scr")
            nc.vector.affine_mul_reduce(
                out=scr.rearrange("p (j t) -> p j t", t=T),
                accum_out=pos_acc[:, rg:rg + 1],
                in0=s.rearrange("p (j t) -> p j t", t=T),
                in1=oh_sb[:, rg, :].unsqueeze(1).broadcast_to((128, YC // T, T)),
                scale=1.0,
                bias=0.0,
            )

        # ---- xx stage: this core's x rows vs all x columns ----
        for g in range(NXG):
            pt = psum.tile([128, N], F32, name="pt_xx", tag="pt")
            for k in range(KC):
                lhsT = xtl_sb[k][:, g * 128:(g + 1) * 128]
                for cb in range(N // NB):
                    nc.tensor.matmul(
                        pt[:, cb * NB:(cb + 1) * NB],
                        lhsT,
                        xt_sb[k][:, cb * NB:(cb + 1) * NB],
                        start=(k == 0),
                        stop=(k == KC - 1),
                    )
            s = sp.tile([128, N], F32, name="s_xx", tag="s")
            nc.scalar.activation(
                out=s, in_=pt, func=AF.Exp, scale=1.0 / TEMP,
                accum_out=xxt_acc[:, g:g + 1],
            )
            scr = scrp.tile([128, N], F32, name="scr_xx", tag="scr")
            nc.vector.affine_mul_reduce(
                out=scr,
                accum_out=xxp_acc[:, g:g + 1],
                in0=s,
                in1=mm_sb[:, g, :],
                scale=1.0,
                bias=0.0,
            )

        nc.sync.dma_start(out=out[:, 0:NRG], in_=pos_acc)
        nc.sync.dma_start(out=out[:, NRG:2 * NRG], in_=tot_acc)
        nc.sync.dma_start(out=out[:, 2 * NRG:2 * NRG + NXG], in_=xxp_acc)
        nc.sync.dma_start(out=out[:, 2 * NRG + NXG:2 * NRG + 2 * NXG], in_=xxt_acc)


def make_in_maps(x, tid):
    """Per-core input dicts. x: [N, D] f32; tid: [N] int."""
    import ml_dtypes
    xt3 = np.ascontiguousarray(x.T).astype(ml_dtypes.bfloat16).reshape(KC, 128, N)
    oh = np.zeros((N, T), np.float32)
    oh[np.arange(N), tid] = 1.0
    oh3 = np.ascontiguousarray(oh.reshape(NRG, 128, T))

    in_maps = []
    for c in range(NCORES):
        xtl = np.ascontiguousarray(xt3[:, :, c * R:(c + 1) * R])
        rows = tid[c * R:(c + 1) * R]
        mm = (rows[:, None] == tid[None, :]).astype(np.float32)
        mm3 = np.ascontiguousarray(mm.reshape(NXG, 128, N))
        in_maps.append({
            "xt": xt3,
            "xtl": xtl,
            "yt": None,  # filled below (depends on y)
            "oh": oh3,
            "mmk": mm3,
        })
    return in_maps


def fill_y_slices(in_maps, y):
    yf = np.ascontiguousarray(y.reshape(T * Q, D))
    for c in range(NCORES):
        import ml_dtypes
        ys = yf[c * YC:(c + 1) * YC]          # [YC, D]
        in_maps[c]["yt"] = np.ascontiguousarray(ys.T).astype(ml_dtypes.bfloat16).reshape(KC, 128, YC)


def combine(outs, x, tid):
    """outs: list of per-core 'out' arrays [128, 36]. Returns loss [1] f32."""
    rxy_pos = np.zeros(N, np.float64)
    rxy_tot = np.zeros(N, np.float64)
    rxx_posf = np.zeros(N, np.float64)
    rxx_tot = np.zeros(N, np.float64)
    for c, o in enumerate(outs):
        o = o.astype(np.float64)
        # xy partials cover all rows; row of (p, rg) is 128*rg + p
        rxy_pos += o[:, 0:NRG].T.reshape(N)
        rxy_tot += o[:, NRG:2 * NRG].T.reshape(N)
        # xx covers this core's rows only
        rxx_posf[c * R:(c + 1) * R] = o[:, 2 * NRG:2 * NRG + NXG].T.reshape(R)
        rxx_tot[c * R:(c + 1) * R] = o[:, 2 * NRG + NXG:2 * NRG + 2 * NXG].T.reshape(R)

    xd = x.astype(np.float64)
    diag = np.exp((xd * xd).sum(axis=1) / TEMP)

    num_i = rxy_pos + 0.5 * (rxx_posf - diag)
    den_i = (rxy_tot - rxy_pos) + (rxx_tot - rxx_posf)

    num_t = np.bincount(tid, weights=num_i, minlength=T)
    den_t = np.bincount(tid, weights=den_i, minlength=T)
    counts = np.bincount(tid, minlength=T)
    present = counts > 0
    loss_t = -np.log(num_t[present] / (den_t[present] + num_t[present]))
    loss = loss_t.sum() / present.sum()
    return np.asarray([loss], dtype=np.float32)


_NC_CACHE = None


def _get_nc():
    global _NC_CACHE
    if _NC_CACHE is None:
        _NC_CACHE = _build_bass()
    return _NC_CACHE


def kernel(x, track_idxs, y, _trace=False):
    x = np.ascontiguousarray(np.asarray(x), dtype=np.float32)
    y = np.ascontiguousarray(np.asarray(y), dtype=np.float32)
    tid = np.asarray(track_idxs).astype(np.int64)

    nc = _get_nc()
    in_maps = make_in_maps(x, tid)
    fill_y_slices(in_maps, y)

    res = bass_utils.run_bass_kernel_spmd(
        nc, in_maps, core_ids=list(range(NCORES)), trace=_trace,
    )
    outs = [r["out"] for r in res.results]
    loss = combine(outs, x, tid)
    if _trace:
        return loss, res
    return loss
